# revision 12
# baseline (speedup 1.0000x reference)
"""Capsule-routing layer on 8 Trainium2 NeuronCores.

Math (per batch b, output slot ot=(o,t), positions n=(c,h,w)):
    pred[b,n,ot] = sum_s Ts[n,ot,s] * x[b,n,s]        (Ts = T[0]*0.1)
    S1[b,ot] = sum_n p[b,n]*pred ;  S2[b,ot] = sum_n p[b,n]*pred^2
    psum[b]  = sum_n p[b,n]
    caps = S1/(psum+eps)
    var  = (S2 - 2*caps*S1 + caps^2*psum)/(psum+eps)   (exact expansion)
    vs = var.sum(t);  p_upd = 1 - vs/(vs.max(o)+eps)

Sharding: tensor-parallel over in_c — 4 channels per core, so each core reads
only its slice of the 65MB transform (4.1MB in bf16) and every per-core
output is a partial sum.  The host adds the 8 per-core partials (and the 4
PE column-strip partials, see below) and finishes the tiny algebra in
float64.

Per-core kernel, per (c,h) iteration (56 total):
  - pred: matmul(lhsT = block-diag sqrt(p)*x [126,112], rhs = Ts [126,288])
          per b-half -> PSUM [112=(w,b8), 288].  sqrt(p) is folded into x so
          sum(p*pred^2) becomes a plain sum of squares.
  - sq:   Square on ScalarE straight from PSUM, or copy+self-mul on VectorE
          (2:1 rotation), PSUM -> SBUF bf16.
  - S2:   constant delta-mask matmul [112,16]^T @ sq accumulates sum over w
          into a persistent PSUM bank.  These M=16 matmuls are issued in
          groups of 4 to distinct PE column-groups (tile_position=(0,32j)),
          with a shared dependency on the group's last square so they land
          back-to-back in the PE stream and execute concurrently.
  - S1:   dense matmul(lhsT = p*x [126,16], rhs = Ts), same 4-way column
          packing; full-density contraction since S1 is linear in x.
All operands stay SBUF-resident; input DMAs are a few large transfers with
small graduated lead-in chunks so the PE starts ~2us into the kernel.
"""

import os

import numpy as np
import ml_dtypes

B, C, O, H, W, S, TT = 16, 32, 32, 14, 14, 9, 9
OT = O * TT          # 288
NCORES = 8
CS = C // NCORES     # 4 in-channels per core
NIT = CS * H         # 56 iterations per core
KP = W * S           # 126 contraction rows (w,s)
MB = W * 8           # 112 output rows (w, b-half)
EPS = 1e-8

DT_NAME = os.environ.get("CAPS_DT", "bfloat16")  # bfloat16 | float32r | float32
SQ_PAT = ("act", "act", "dve")                   # square-engine rotation

_cache = {}
last_perf = None


def _np_dt():
    return {
        "bfloat16": ml_dtypes.bfloat16,
        "float32r": np.float32,
        "float32": np.float32,
    }[DT_NAME]


def _build():
    from concourse.bacc import Bacc
    from concourse.bass import _add_dep_helper
    from concourse.tile import TileContext
    import concourse.mybir as mybir

    DT = getattr(mybir.dt, DT_NAME)
    F32 = mybir.dt.float32
    nc = Bacc("TRN2", target_bir_lowering=False)
    ts_d = nc.dram_tensor("ts_in", [CS, KP, H * OT], DT, kind="ExternalInput")
    xbd_d = nc.dram_tensor("xbd_in", [CS, KP, H * 2 * MB], DT, kind="ExternalInput")
    xp_d = nc.dram_tensor("xp_in", [CS, KP, H * B], DT, kind="ExternalInput")
    dm_d = nc.dram_tensor("dm_in", [MB, 2 * B], DT, kind="ExternalInput")
    so_d = nc.dram_tensor("s_out", [128, 2 * OT], F32, kind="ExternalOutput")

    with TileContext(nc) as tc:
        with tc.tile_pool(name="wgt", bufs=1) as wp, \
             tc.tile_pool(name="sq", bufs=8) as sqp, \
             tc.tile_pool(name="pc", bufs=4) as pcp, \
             tc.tile_pool(name="pp", bufs=4, space="PSUM") as pp, \
             tc.tile_pool(name="acc", bufs=1, space="PSUM") as accp, \
             tc.tile_pool(name="outp", bufs=1) as op:
            ts_sb = [wp.tile([KP, H * OT], DT, tag=f"ts{c}", name=f"ts{c}")
                     for c in range(CS)]
            xbd_sb = [wp.tile([KP, H * 2 * MB], DT, tag=f"xbd{c}", name=f"xbd{c}")
                      for c in range(CS)]
            xp_sb = [wp.tile([KP, H * B], DT, tag=f"xp{c}", name=f"xp{c}")
                     for c in range(CS)]
            dm_sb = wp.tile([MB, 2 * B], DT, tag="dm", name="dm")

            def dma_h_range(c, h0, h1):
                nc.sync.dma_start(out=xbd_sb[c][:, h0 * 2 * MB:h1 * 2 * MB],
                                  in_=xbd_d[c][:, h0 * 2 * MB:h1 * 2 * MB])
                nc.sync.dma_start(out=xp_sb[c][:, h0 * B:h1 * B],
                                  in_=xp_d[c][:, h0 * B:h1 * B])
                nc.sync.dma_start(out=ts_sb[c][:, h0 * OT:h1 * OT],
                                  in_=ts_d[c][:, h0 * OT:h1 * OT])

            # graduated lead-in: tiny first chunks so the PE starts early
            dma_h_range(0, 0, 2)
            nc.sync.dma_start(out=dm_sb, in_=dm_d[:])
            dma_h_range(0, 2, 6)
            dma_h_range(0, 6, H)
            dma_h_range(1, 0, 7)
            dma_h_range(1, 7, H)
            for c in range(2, CS):
                dma_h_range(c, 0, H)

            # column-strip partial accumulators: strip j holds partitions
            # [32j, 32j+16); host sums the 4 strips
            s1_ps = accp.tile([128, OT], F32, tag="s1")
            s2_ps = accp.tile([128, OT], F32, tag="s2")

            n_d = 2 * NIT
            n_s = NIT
            d_idx = 0
            s_idx = 0
            s1_pending = []
            d_pending = []

            def flush_s1():
                # chain the group so members issue consecutively on PE
                nonlocal s_idx
                prev = None
                for (xp_v, ts_v) in s1_pending:
                    j = s_idx % 4
                    mm = nc.tensor.matmul(
                        s1_ps[32 * j:32 * j + B, :], xp_v, ts_v,
                        start=(s_idx < 4), stop=(s_idx >= n_s - 4),
                        tile_position=(0, 32 * j),
                        skip_group_check=True)
                    if prev is not None:
                        _add_dep_helper(
                            mm.ins, prev.ins, sync=True,
                            reason="chain S1 group")
                    prev = mm
                    s_idx += 1
                s1_pending.clear()

            def flush_d():
                # tie each delta-MM of the group to the group's last square
                # so all 4 become ready together and issue back-to-back
                nonlocal d_idx
                last_sq_inst = d_pending[-1][2] if d_pending else None
                for (sq, hf, sq_inst) in d_pending:
                    j = d_idx % 4
                    mm = nc.tensor.matmul(
                        s2_ps[32 * j:32 * j + B, :],
                        dm_sb[:, hf * B:(hf + 1) * B], sq,
                        start=(d_idx < 4), stop=(d_idx >= n_d - 4),
                        tile_position=(0, 32 * j),
                        skip_group_check=True)
                    if last_sq_inst is not None and sq_inst is not last_sq_inst:
                        _add_dep_helper(
                            mm.ins, last_sq_inst.ins, sync=True,
                            reason="group delta-MMs on last square")
                    d_idx += 1
                d_pending.clear()

            # Emission is software-pipelined one block (2 iterations) deep:
            # block k's 4 delta-MMs are emitted after block k+1's preds and
            # squares so they are all ready when their PE-stream turn comes.
            for c in range(CS):
                for h0 in range(0, H, 2):
                    if len(s1_pending) == 4:
                        flush_s1()
                    sqs = []
                    for dh in range(2):
                        h = h0 + dh
                        it = c * H + h
                        ts_v = ts_sb[c][:, h * OT:(h + 1) * OT]
                        s1_pending.append(
                            (xp_sb[c][:, h * B:(h + 1) * B], ts_v))
                        for hf in range(2):
                            base = h * 2 * MB + hf * MB
                            pred = pp.tile([MB, OT], F32, tag="pred")
                            nc.tensor.matmul(
                                pred, xbd_sb[c][:, base:base + MB], ts_v,
                                start=True, stop=True)
                            sq = sqp.tile([MB, OT], DT, tag="sq")
                            if SQ_PAT[(it * 2 + hf) % len(SQ_PAT)] == "act":
                                sq_inst = nc.scalar.square(sq, pred)
                            else:
                                # VectorE may read PSUM only once per op:
                                # copy to SBUF (bf16), then self-multiply
                                pcc = pcp.tile([MB, OT], DT, tag="pc")
                                nc.vector.tensor_copy(pcc, pred)
                                sq_inst = nc.vector.tensor_mul(sq, pcc, pcc)
                            sqs.append((sq, hf, sq_inst))
                    flush_d()
                    d_pending.extend(sqs)
            flush_d()
            flush_s1()

            s_o = op.tile([128, 2 * OT], F32, tag="so", name="so")
            nc.vector.tensor_copy(s_o[:, 0:OT], s1_ps)
            nc.scalar.copy(s_o[:, OT:2 * OT], s2_ps)
            nc.sync.dma_start(out=so_d[:], in_=s_o)
    nc.finalize()
    return nc


def _prep_inputs(x, p, Tm):
    """Host-side re-layout into the exact SBUF layouts the kernel uses."""
    ndt = _np_dt()
    Ts_all = (Tm[0] * np.float32(0.1)).astype(np.float32)  # (c,o,h,w,t,s)
    sqrt_p = np.sqrt(p)                                    # (b,c,h,w)
    xq = x * sqrt_p[..., None]                             # (b,c,h,w,s)
    xp = x * p[..., None]                                  # (b,c,h,w,s)

    dm = np.zeros((W, 8, 2, B), np.float32)
    for bl in range(8):
        dm[:, bl, 0, bl] = 1.0
        dm[:, bl, 1, 8 + bl] = 1.0
    dm = np.ascontiguousarray(dm.reshape(MB, 2 * B)).astype(ndt)

    in_maps = []
    for core in range(NCORES):
        c0 = core * CS
        tsc = Ts_all[c0:c0 + CS].transpose(0, 3, 5, 2, 1, 4)  # (c,w,s,h,o,t)
        tsc = np.ascontiguousarray(tsc.reshape(CS, KP, H * OT)).astype(ndt)

        xqc = xq[:, c0:c0 + CS].transpose(1, 3, 4, 2, 0)      # (c,w,s,h,b)
        xbd = np.zeros((CS, W, S, H, 2, W, 8), np.float32)
        for w in range(W):
            xbd[:, w, :, :, 0, w, :] = xqc[:, w, :, :, 0:8]
            xbd[:, w, :, :, 1, w, :] = xqc[:, w, :, :, 8:16]
        xbd = np.ascontiguousarray(
            xbd.reshape(CS, KP, H * 2 * MB)).astype(ndt)

        xpc = xp[:, c0:c0 + CS].transpose(1, 3, 4, 2, 0)      # (c,w,s,h,b)
        xpc = np.ascontiguousarray(xpc.reshape(CS, KP, H * B)).astype(ndt)

        in_maps.append({
            "ts_in": tsc, "xbd_in": xbd, "xp_in": xpc, "dm_in": dm,
        })
    return in_maps


def kernel(x=None, p=None, epoch=None, T=None, **_unused):
    global last_perf
    x = np.asarray(x, dtype=np.float32)
    p = np.asarray(p, dtype=np.float32)
    Tm = np.asarray(T, dtype=np.float32)

    from concourse.bass_utils import run_bass_kernel_spmd

    if "nc" not in _cache:
        _cache["nc"] = _build()
    nc = _cache["nc"]

    in_maps = _prep_inputs(x, p, Tm)
    res = run_bass_kernel_spmd(
        nc, in_maps, core_ids=list(range(NCORES)),
        trace=bool(os.environ.get("CAPS_TRACE")),
    )
    last_perf = res

    St = np.zeros((128, 2 * OT), np.float64)
    for r in res.results:
        St += r["s_out"].astype(np.float64)
    St = St.reshape(4, 32, 2 * OT)[:, :B].sum(axis=0)     # sum the 4 strips
    S1 = St[:, :OT].reshape(B, O, TT)
    S2 = St[:, OT:].reshape(B, O, TT)
    psum = p.sum(axis=(1, 2, 3)).astype(np.float64)[:, None, None]
    den = psum + EPS
    caps = S1 / den
    var = (S2 - 2.0 * caps * S1 + caps * caps * psum) / den
    vs = var.sum(axis=2)                                  # (b, o)
    p_upd = 1.0 - vs / (vs.max(axis=1, keepdims=True) + EPS)
    return caps.astype(np.float32), p_upd.astype(np.float32)


# revision 13
# speedup vs baseline: 1.0069x; 1.0069x over previous
"""Capsule-routing layer on 8 Trainium2 NeuronCores.

Math (per batch b, output slot ot=(o,t), positions n=(c,h,w)):
    pred[b,n,ot] = sum_s Ts[n,ot,s] * x[b,n,s]        (Ts = T[0]*0.1)
    S1[b,ot] = sum_n p[b,n]*pred ;  S2[b,ot] = sum_n p[b,n]*pred^2
    psum[b]  = sum_n p[b,n]
    caps = S1/(psum+eps)
    var  = (S2 - 2*caps*S1 + caps^2*psum)/(psum+eps)   (exact expansion)
    vs = var.sum(t);  p_upd = 1 - vs/(vs.max(o)+eps)

Sharding: tensor-parallel over in_c — 4 channels per core, so each core reads
only its slice of the 65MB transform (4.1MB in bf16) and every per-core
output is a partial sum.  The host adds the 8 per-core partials (and the 4
PE column-strip partials, see below) and finishes the tiny algebra in
float64.

Per-core kernel, per (c,h) iteration (56 total):
  - pred: matmul(lhsT = block-diag sqrt(p)*x [126,112], rhs = Ts [126,288])
          per b-half -> PSUM [112=(w,b8), 288].  sqrt(p) is folded into x so
          sum(p*pred^2) becomes a plain sum of squares.
  - sq:   Square on ScalarE straight from PSUM, or copy+self-mul on VectorE
          (2:1 rotation), PSUM -> SBUF bf16.
  - S2:   constant delta-mask matmul [112,16]^T @ sq accumulates sum over w
          into a persistent PSUM bank.  These M=16 matmuls are issued in
          groups of 4 to distinct PE column-groups (tile_position=(0,32j)),
          with a shared dependency on the group's last square so they land
          back-to-back in the PE stream and execute concurrently.
  - S1:   dense matmul(lhsT = p*x [126,16], rhs = Ts), same 4-way column
          packing; full-density contraction since S1 is linear in x.
All operands stay SBUF-resident; input DMAs are a few large transfers with
small graduated lead-in chunks so the PE starts ~2us into the kernel.
"""

import os

import numpy as np
import ml_dtypes

B, C, O, H, W, S, TT = 16, 32, 32, 14, 14, 9, 9
OT = O * TT          # 288
NCORES = 8
CS = C // NCORES     # 4 in-channels per core
NIT = CS * H         # 56 iterations per core
KP = W * S           # 126 contraction rows (w,s)
MB = W * 8           # 112 output rows (w, b-half)
EPS = 1e-8

DT_NAME = os.environ.get("CAPS_DT", "bfloat16")  # bfloat16 | float32r | float32
SQ_PAT = ("act", "act", "dve")                   # square-engine rotation

_cache = {}
last_perf = None


def _np_dt():
    return {
        "bfloat16": ml_dtypes.bfloat16,
        "float32r": np.float32,
        "float32": np.float32,
    }[DT_NAME]


def _build():
    from concourse.bacc import Bacc
    from concourse.bass import _add_dep_helper
    from concourse.tile import TileContext
    import concourse.mybir as mybir

    DT = getattr(mybir.dt, DT_NAME)
    F32 = mybir.dt.float32
    nc = Bacc("TRN2", target_bir_lowering=False)
    ts_d = nc.dram_tensor("ts_in", [CS, KP, H * OT], DT, kind="ExternalInput")
    xbd_d = nc.dram_tensor("xbd_in", [CS, KP, H * 2 * MB], DT, kind="ExternalInput")
    xp_d = nc.dram_tensor("xp_in", [CS, KP, H * B], DT, kind="ExternalInput")
    dm_d = nc.dram_tensor("dm_in", [MB, 2 * B], DT, kind="ExternalInput")
    so_d = nc.dram_tensor("s_out", [128, 2 * OT], F32, kind="ExternalOutput")

    with TileContext(nc) as tc:
        with tc.tile_pool(name="wgt", bufs=1) as wp, \
             tc.tile_pool(name="sq", bufs=10) as sqp, \
             tc.tile_pool(name="pc", bufs=4) as pcp, \
             tc.tile_pool(name="pp", bufs=4, space="PSUM") as pp, \
             tc.tile_pool(name="acc", bufs=1, space="PSUM") as accp, \
             tc.tile_pool(name="outp", bufs=1) as op:
            ts_sb = [wp.tile([KP, H * OT], DT, tag=f"ts{c}", name=f"ts{c}")
                     for c in range(CS)]
            xbd_sb = [wp.tile([KP, H * 2 * MB], DT, tag=f"xbd{c}", name=f"xbd{c}")
                      for c in range(CS)]
            xp_sb = [wp.tile([KP, H * B], DT, tag=f"xp{c}", name=f"xp{c}")
                     for c in range(CS)]
            dm_sb = wp.tile([MB, 2 * B], DT, tag="dm", name="dm")

            def dma_h_range(c, h0, h1):
                nc.sync.dma_start(out=xbd_sb[c][:, h0 * 2 * MB:h1 * 2 * MB],
                                  in_=xbd_d[c][:, h0 * 2 * MB:h1 * 2 * MB])
                nc.sync.dma_start(out=xp_sb[c][:, h0 * B:h1 * B],
                                  in_=xp_d[c][:, h0 * B:h1 * B])
                nc.sync.dma_start(out=ts_sb[c][:, h0 * OT:h1 * OT],
                                  in_=ts_d[c][:, h0 * OT:h1 * OT])

            # graduated lead-in: tiny first chunks so the PE starts early
            dma_h_range(0, 0, 2)
            nc.sync.dma_start(out=dm_sb, in_=dm_d[:])
            dma_h_range(0, 2, 6)
            dma_h_range(0, 6, H)
            dma_h_range(1, 0, 7)
            dma_h_range(1, 7, H)
            for c in range(2, CS):
                dma_h_range(c, 0, H)

            # column-strip partial accumulators: strip j holds partitions
            # [32j, 32j+16); host sums the 4 strips
            s1_ps = accp.tile([128, OT], F32, tag="s1")
            s2_ps = accp.tile([128, OT], F32, tag="s2")

            n_d = 2 * NIT
            n_s = NIT
            d_idx = 0
            s_idx = 0
            s1_pending = []
            d_pending = []

            def flush_s1():
                # chain the group so members issue consecutively on PE
                nonlocal s_idx
                prev = None
                for (xp_v, ts_v) in s1_pending:
                    j = s_idx % 4
                    mm = nc.tensor.matmul(
                        s1_ps[32 * j:32 * j + B, :], xp_v, ts_v,
                        start=(s_idx < 4), stop=(s_idx >= n_s - 4),
                        tile_position=(0, 32 * j),
                        skip_group_check=True)
                    if prev is not None:
                        _add_dep_helper(
                            mm.ins, prev.ins, sync=True,
                            reason="chain S1 group")
                    prev = mm
                    s_idx += 1
                s1_pending.clear()

            def flush_d():
                # tie each delta-MM of the group to the group's last square
                # so all 4 become ready together and issue back-to-back
                nonlocal d_idx
                last_sq_inst = d_pending[-1][2] if d_pending else None
                for (sq, hf, sq_inst) in d_pending:
                    j = d_idx % 4
                    mm = nc.tensor.matmul(
                        s2_ps[32 * j:32 * j + B, :],
                        dm_sb[:, hf * B:(hf + 1) * B], sq,
                        start=(d_idx < 4), stop=(d_idx >= n_d - 4),
                        tile_position=(0, 32 * j),
                        skip_group_check=True)
                    if last_sq_inst is not None and sq_inst is not last_sq_inst:
                        _add_dep_helper(
                            mm.ins, last_sq_inst.ins, sync=True,
                            reason="group delta-MMs on last square")
                    d_idx += 1
                d_pending.clear()

            # Emission is software-pipelined one block (2 iterations) deep:
            # block k's 4 delta-MMs are emitted after block k+1's preds and
            # squares so they are all ready when their PE-stream turn comes.
            for c in range(CS):
                for h0 in range(0, H, 2):
                    if len(s1_pending) == 4:
                        flush_s1()
                    sqs = []
                    for dh in range(2):
                        h = h0 + dh
                        it = c * H + h
                        ts_v = ts_sb[c][:, h * OT:(h + 1) * OT]
                        s1_pending.append(
                            (xp_sb[c][:, h * B:(h + 1) * B], ts_v))
                        for hf in range(2):
                            base = h * 2 * MB + hf * MB
                            pred = pp.tile([MB, OT], F32, tag="pred")
                            nc.tensor.matmul(
                                pred, xbd_sb[c][:, base:base + MB], ts_v,
                                start=True, stop=True)
                            sq = sqp.tile([MB, OT], DT, tag="sq")
                            if SQ_PAT[(it * 2 + hf) % len(SQ_PAT)] == "act":
                                sq_inst = nc.scalar.square(sq, pred)
                            else:
                                # VectorE may read PSUM only once per op:
                                # copy to SBUF (bf16), then self-multiply
                                pcc = pcp.tile([MB, OT], DT, tag="pc")
                                nc.vector.tensor_copy(pcc, pred)
                                sq_inst = nc.vector.tensor_mul(sq, pcc, pcc)
                            sqs.append((sq, hf, sq_inst))
                    flush_d()
                    d_pending.extend(sqs)
            flush_d()
            flush_s1()

            s_o = op.tile([128, 2 * OT], F32, tag="so", name="so")
            nc.vector.tensor_copy(s_o[:, 0:OT], s1_ps)
            nc.scalar.copy(s_o[:, OT:2 * OT], s2_ps)
            nc.sync.dma_start(out=so_d[:], in_=s_o)
    nc.finalize()
    return nc


def _prep_inputs(x, p, Tm):
    """Host-side re-layout into the exact SBUF layouts the kernel uses."""
    ndt = _np_dt()
    Ts_all = (Tm[0] * np.float32(0.1)).astype(np.float32)  # (c,o,h,w,t,s)
    sqrt_p = np.sqrt(p)                                    # (b,c,h,w)
    xq = x * sqrt_p[..., None]                             # (b,c,h,w,s)
    xp = x * p[..., None]                                  # (b,c,h,w,s)

    dm = np.zeros((W, 8, 2, B), np.float32)
    for bl in range(8):
        dm[:, bl, 0, bl] = 1.0
        dm[:, bl, 1, 8 + bl] = 1.0
    dm = np.ascontiguousarray(dm.reshape(MB, 2 * B)).astype(ndt)

    in_maps = []
    for core in range(NCORES):
        c0 = core * CS
        tsc = Ts_all[c0:c0 + CS].transpose(0, 3, 5, 2, 1, 4)  # (c,w,s,h,o,t)
        tsc = np.ascontiguousarray(tsc.reshape(CS, KP, H * OT)).astype(ndt)

        xqc = xq[:, c0:c0 + CS].transpose(1, 3, 4, 2, 0)      # (c,w,s,h,b)
        xbd = np.zeros((CS, W, S, H, 2, W, 8), np.float32)
        for w in range(W):
            xbd[:, w, :, :, 0, w, :] = xqc[:, w, :, :, 0:8]
            xbd[:, w, :, :, 1, w, :] = xqc[:, w, :, :, 8:16]
        xbd = np.ascontiguousarray(
            xbd.reshape(CS, KP, H * 2 * MB)).astype(ndt)

        xpc = xp[:, c0:c0 + CS].transpose(1, 3, 4, 2, 0)      # (c,w,s,h,b)
        xpc = np.ascontiguousarray(xpc.reshape(CS, KP, H * B)).astype(ndt)

        in_maps.append({
            "ts_in": tsc, "xbd_in": xbd, "xp_in": xpc, "dm_in": dm,
        })
    return in_maps


def kernel(x=None, p=None, epoch=None, T=None, **_unused):
    global last_perf
    x = np.asarray(x, dtype=np.float32)
    p = np.asarray(p, dtype=np.float32)
    Tm = np.asarray(T, dtype=np.float32)

    from concourse.bass_utils import run_bass_kernel_spmd

    if "nc" not in _cache:
        _cache["nc"] = _build()
    nc = _cache["nc"]

    in_maps = _prep_inputs(x, p, Tm)
    res = run_bass_kernel_spmd(
        nc, in_maps, core_ids=list(range(NCORES)),
        trace=bool(os.environ.get("CAPS_TRACE")),
    )
    last_perf = res

    St = np.zeros((128, 2 * OT), np.float64)
    for r in res.results:
        St += r["s_out"].astype(np.float64)
    St = St.reshape(4, 32, 2 * OT)[:, :B].sum(axis=0)     # sum the 4 strips
    S1 = St[:, :OT].reshape(B, O, TT)
    S2 = St[:, OT:].reshape(B, O, TT)
    psum = p.sum(axis=(1, 2, 3)).astype(np.float64)[:, None, None]
    den = psum + EPS
    caps = S1 / den
    var = (S2 - 2.0 * caps * S1 + caps * caps * psum) / den
    vs = var.sum(axis=2)                                  # (b, o)
    p_upd = 1.0 - vs / (vs.max(axis=1, keepdims=True) + EPS)
    return caps.astype(np.float32), p_upd.astype(np.float32)
